# revision 25
# baseline (speedup 1.0000x reference)
"""Masked multi-head attention (B=2, N=2048, C=1152, H=16, HD=72) on 8 trn2 cores.

Sharding: core c handles batch b = c//4 and heads 4*(c%4) .. 4*(c%4)+4
(data parallel on B, tensor parallel on heads). Each core computes a partial
output projection over its 288 head-channels; the host sums the 4 partials
per batch (TP unshard). b_proj/4 is added on each core so the sum carries the
full bias.

Per-core kernel layout (all fp32, matmuls run as float32r):
  xT   [1152, 2048]  x[b] transposed (contraction dim on partitions)
  qk proj -> QM/KB [73, 4, 2048]: rows 0..71 = per-head qT (mask*scale folded
             in) / kT; row 72 = mask row (q side) / -1e8*(1-mask) (key side).
             Scores matmul then contracts K=73 so the additive mask bias
             bk[k]*m[q] comes out of the PE for free, and masked q columns
             get s=0 -> exp(0)=1 -> uniform attention = reference semantics
             of softmax over an all(-1e8) row.
  v proj  -> V [128, 16, 292] (keys on partitions, 4*(72+1) cols: col 72 of
             each head is 1.0 so the AV matmul also produces the softmax
             denominator).
  scores/exp/AV per (head, 1024-wide q chunk), normalize via DVE reciprocal
             + K=1 PE broadcast matmul, proj accumulates 4 head chunks.
"""

import numpy as np

import concourse.bass as bass
import concourse.mybir as mybir
import concourse.tile as tile
from concourse import bacc
from concourse.bass_utils import run_bass_kernel_spmd

B, N, C = 2, 2048, 1152
H = 16
HD = C // H  # 72
HL = 4  # heads per core
SCALE = HD ** -0.5
B0 = 100007936.0  # fp32r-exact rounding of 1e8, multiple of 8
KD = HD + 1  # 73: contraction + key-bias row
VW = 97  # V slot width per head: v(72) + pad + ones col at 96 (aligned D row)
NT = N // 128  # 16 key chunks
CC = C // 128  # 9 contraction chunks
QW = 1024  # q-chunk width for the exp pass
NQ = N // QW  # 2

F32 = mybir.dt.float32
F32R = mybir.dt.float32r


def build_nc(debug=False):
    nc = bacc.Bacc(None, target_bir_lowering=False)

    xT = nc.dram_tensor("xT", [C, N], F32R, kind="ExternalInput")
    wqkT = nc.dram_tensor("wqkT", [C, 2 * HL * HD], F32R, kind="ExternalInput")
    wvT = nc.dram_tensor("wvT", [C, HL * HD], F32R, kind="ExternalInput")
    wpT = nc.dram_tensor("wpT", [HL * HD, C], F32R, kind="ExternalInput")
    mrow = nc.dram_tensor("mrow", [1, N], F32R, kind="ExternalInput")
    bkrow = nc.dram_tensor("bkrow", [1, N], F32R, kind="ExternalInput")
    abrow = nc.dram_tensor("abrow", [1, N], F32R, kind="ExternalInput")
    bq = nc.dram_tensor("bq", [C], F32, kind="ExternalInput")
    outP = nc.dram_tensor("outP", [C, N], F32, kind="ExternalOutput")
    if debug:
        qm_out = nc.dram_tensor("qm_out", [KD, HL, N], F32, kind="ExternalOutput")
        kb_out = nc.dram_tensor("kb_out", [KD, HL, N], F32, kind="ExternalOutput")
        v_out = nc.dram_tensor("v_out", [128, NT, HL * VW], F32, kind="ExternalOutput")
        ao_out = nc.dram_tensor("ao_out", [HD, HL, N], F32, kind="ExternalOutput")
        a_out = nc.dram_tensor("a_out", [128, QW], F32, kind="ExternalOutput")

    with tile.TileContext(nc) as tc:
        with (
            tc.tile_pool(name="persist", bufs=1) as persist,
        ):
            QM = persist.tile([KD, HL, N], F32R)   # q-side, mask*scale folded
            KB = persist.tile([KD, HL, N], F32R)   # k-side + bias row
            V = persist.tile([128, NT, HL * VW], F32R)
            AO = persist.tile([HD, HL, N], F32R)   # normalized attention out
            ones1 = persist.tile([1, 128], F32R)
            ones1f = persist.tile([1, 128], F32)
            ones128 = persist.tile([128, 1], F32)
            bq_sb = persist.tile([128, CC], F32)

            nc.vector.memset(ones1f, 1.0)
            nc.vector.memset(ones128, 1.0)
            nc.vector.tensor_copy(ones1, ones1f)
            nc.sync.dma_start(bq_sb, bq.rearrange("(o p) -> p o", p=128))
            # bias row 72: QM=m_q, KB=-B0(1-m_k): key-mask bias for unmasked q.
            # Masked-q columns are handled by mm2 (K=1 psum-accumulate of
            # -B0(1-m_q), which quantizes the psum-resident score exactly like
            # the reference's fp32 add of -1e8) + a DVE add-back of +B0(1-m_q)
            # before the exp.
            for h in range(HL):
                nc.sync.dma_start(QM[HD : HD + 1, h, :], mrow[:])
                nc.sync.dma_start(KB[HD : HD + 1, h, :], bkrow[:])
            # V pad + ones columns (cols 72..96 of each head slot) = 1.0
            vtail = V.rearrange("p t (h w) -> p t h w", h=HL)[:, :, :, HD:]
            nc.vector.tensor_copy(
                vtail, ones128.to_broadcast(vtail.shape)
            )

            # ---------- phase 0+1: replicated mask + projections ----------
            with (
                tc.tile_pool(name="ph1", bufs=12) as ph1,
                tc.tile_pool(name="wpool", bufs=1) as wpool,
                tc.tile_pool(name="ps_v", bufs=2, space="PSUM") as ps_v,
                tc.tile_pool(name="ps_qk", bufs=3, space="PSUM") as ps_qk,
            ):
                wqk_sb = wpool.tile([128, CC, 2 * HL * HD], F32R)
                wv_sb = wpool.tile([128, CC, HL * HD], F32R)
                nc.sync.dma_start(
                    wqk_sb, wqkT.rearrange("(o p) m -> p o m", p=128)
                )
                nc.sync.dma_start(wv_sb, wvT.rearrange("(o p) m -> p o m", p=128))

                for nq in range(4):  # 512-token chunks
                    q0 = nq * 512
                    xts = []
                    for cc in range(CC):
                        xt = ph1.tile(
                            [128, 512], F32R, tag="xts", name=f"x_{nq}_{cc}"
                        )
                        nc.sync.dma_start(
                            xt, xT[cc * 128 : (cc + 1) * 128, q0 : q0 + 512]
                        )
                        xts.append(xt)
                    # v projection: tokens on partitions
                    for t in range(4):
                        pv = ps_v.tile([128, HL * HD], F32, tag="pv")
                        for cc in range(CC):
                            nc.tensor.matmul(
                                pv,
                                (xts[cc][:, t * 128 : (t + 1) * 128]),
                                (wv_sb[:, cc, :]),
                                start=(cc == 0),
                                stop=(cc == CC - 1),
                            )
                        kc = nq * 4 + t
                        nc.vector.tensor_copy(
                            V[:, kc, :].rearrange("p (h w) -> p h w", h=HL)[
                                :, :, :HD
                            ],
                            pv.rearrange("p (h w) -> p h w", h=HL),
                        )
                    # q/k projection: channels on partitions, one head per tile
                    for is_q in (True, False):
                        for h in range(HL):
                            r0 = (0 if is_q else HL * HD) + h * HD
                            pqk = ps_qk.tile([HD, 512], F32, tag="pqk")
                            for cc in range(CC):
                                nc.tensor.matmul(
                                    pqk,
                                    (wqk_sb[:, cc, r0 : r0 + HD]),
                                    (xts[cc]),
                                    start=(cc == 0),
                                    stop=(cc == CC - 1),
                                )
                            if is_q:
                                nc.vector.tensor_scalar_mul(
                                    QM[:HD, h, q0 : q0 + 512], pqk, SCALE
                                )
                            else:
                                nc.vector.tensor_copy(
                                    KB[:HD, h, q0 : q0 + 512], pqk
                                )

            # ---------- phase 2: attention ----------
            with (
                tc.tile_pool(name="apool", bufs=4) as apool,
                tc.tile_pool(name="spool", bufs=4) as spool,
                tc.tile_pool(name="small", bufs=2) as small,
                tc.tile_pool(name="ph2row", bufs=1) as ph2row,
                tc.tile_pool(name="ps_s", bufs=2, space="PSUM") as ps_s,
                tc.tile_pool(name="ps_av", bufs=2, space="PSUM") as ps_av,
                tc.tile_pool(name="ps_rep", bufs=1, space="PSUM") as ps_rep,
            ):
                bkrow_sb = ph2row.tile([1, N], F32R)
                abrow_sb = ph2row.tile([1, N], F32R)
                Rqpos = ph2row.tile([128, N], F32)
                nc.sync.dma_start(bkrow_sb, bkrow[:])
                nc.sync.dma_start(abrow_sb, abrow[:])
                for i in range(4):
                    prq = ps_av.tile([128, 512], F32, tag="rq", name=f"rq{i}", bufs=1)
                    nc.tensor.matmul(
                        prq,
                        ones1[:1, :],
                        abrow_sb[:, i * 512 : (i + 1) * 512],
                        start=True,
                        stop=True,
                    )
                    nc.scalar.copy(Rqpos[:, i * 512 : (i + 1) * 512], prq)
                for h in range(HL):
                    for qc in range(NQ):
                        q0 = qc * QW
                        pavs = [
                            ps_av.tile(
                                [VW, 512], F32, tag="pav", name=f"pav_{h}_{qc}_{i}"
                            )
                            for i in range(2)
                        ]
                        for kc in range(NT):
                            ps = ps_s.tile([128, QW], F32, tag="ps")
                            for qh in range(2):
                                qs = q0 + qh * 512
                                nc.tensor.matmul(
                                    ps[:, qh * 512 : (qh + 1) * 512],
                                    (KB[:, h, kc * 128 : (kc + 1) * 128]),
                                    (QM[:, h, qs : qs + 512]),
                                    start=True,
                                    stop=False,
                                )
                                # quantizer: psum += -B0*(1-m_q) (rounds like ref)
                                nc.tensor.matmul(
                                    ps[:, qh * 512 : (qh + 1) * 512],
                                    ones1[:1, :],
                                    bkrow_sb[:, qs : qs + 512],
                                    start=False,
                                    stop=True,
                                )
                            st = spool.tile([128, QW], F32, tag="st")
                            nc.vector.tensor_tensor(
                                st, ps, Rqpos[:, q0 : q0 + QW],
                                mybir.AluOpType.add,
                            )
                            a = apool.tile([128, QW], F32R, tag="A")
                            nc.scalar.activation(
                                a, st, mybir.ActivationFunctionType.Exp
                            )
                            if debug and h == 0 and qc == 0 and kc == 0:
                                nc.sync.dma_start(a_out[:], a.bitcast(F32))
                            for qh in range(2):
                                nc.tensor.matmul(
                                    pavs[qh],
                                    (V[:, kc, h * VW : (h + 1) * VW]),
                                    (a[:, qh * 512 : (qh + 1) * 512]),
                                    start=(kc == 0),
                                    stop=(kc == NT - 1),
                                )
                        for qh in range(2):
                            qs = q0 + qh * 512
                            recf = small.tile([1, 512], F32, tag="recf")
                            nc.vector.reciprocal(recf, pavs[qh][96:97, :])
                            rec = small.tile([1, 512], F32R, tag="rec")
                            nc.vector.tensor_copy(rec, recf)
                            prep = ps_rep.tile([HD, 512], F32, tag="prep")
                            nc.tensor.matmul(
                                prep,
                                (ones1[:1, :HD]),
                                (rec),
                                start=True,
                                stop=True,
                            )
                            rep_sb = small.tile([HD, 512], F32, tag="repsb")
                            nc.vector.tensor_copy(rep_sb, prep)
                            nc.vector.tensor_tensor(
                                AO[:, h, qs : qs + 512],
                                pavs[qh][:HD, :],
                                rep_sb,
                                mybir.AluOpType.mult,
                            )

            if debug:
                nc.sync.dma_start(qm_out[:], QM.bitcast(F32))
                nc.sync.dma_start(kb_out[:], KB.bitcast(F32))
                nc.sync.dma_start(v_out[:], V.bitcast(F32))
                nc.sync.dma_start(ao_out[:], AO.bitcast(F32))

            # ---------- phase 3: output projection ----------
            with (
                tc.tile_pool(name="ph3", bufs=1) as ph3,
                tc.tile_pool(name="opool", bufs=3) as opool,
                tc.tile_pool(name="ps_o", bufs=2, space="PSUM") as ps_o,
            ):
                wp_sb = ph3.tile([HD, HL, C], F32R)
                nc.sync.dma_start(
                    wp_sb, wpT.rearrange("(h p) m -> p h m", p=HD)
                )
                for mo in range(CC):
                    for qc in range(4):
                        q0 = qc * 512
                        po = ps_o.tile([128, 512], F32, tag="po")
                        for h in range(HL):
                            nc.tensor.matmul(
                                po,
                                (wp_sb[:, h, mo * 128 : (mo + 1) * 128]),
                                (AO[:, h, q0 : q0 + 512]),
                                start=(h == 0),
                                stop=(h == HL - 1),
                            )
                        ot = opool.tile([128, 512], F32, tag="ot")
                        nc.vector.tensor_scalar_add(ot, po, bq_sb[:, mo : mo + 1])
                        nc.sync.dma_start(
                            outP[mo * 128 : (mo + 1) * 128, q0 : q0 + 512], ot
                        )

    nc.compile()
    return nc


_NC = None


def _get_nc():
    global _NC
    if _NC is None:
        _NC = build_nc()
    return _NC


def round_fp32r(a):
    """Round fp32 to the fp32r memory format (mantissa truncated to 11 bits,
    round-to-nearest) — matches walrus's fp32_to_fp32r."""
    b = np.ascontiguousarray(a, dtype=np.float32).view(np.uint32)
    b = ((b.astype(np.uint64) + 0x800) & 0xFFFFF000).astype(np.uint32)
    return b.view(np.float32)


def make_in_maps(x, attn_mask, w_qkv, w_proj, b_proj):
    in_maps = []
    for core in range(8):
        b = core // 4
        hs = (core % 4) * HL  # first head
        qrows = w_qkv[hs * HD : (hs + HL) * HD, :]
        krows = w_qkv[C + hs * HD : C + (hs + HL) * HD, :]
        vrows = w_qkv[2 * C + hs * HD : 2 * C + (hs + HL) * HD, :]
        m = attn_mask[b].astype(np.float32)
        in_maps.append(
            {
                "xT": round_fp32r(x[b].T),
                "wqkT": round_fp32r(np.concatenate([qrows, krows], axis=0).T),
                "wvT": round_fp32r(vrows.T),
                "wpT": round_fp32r(w_proj[:, hs * HD : (hs + HL) * HD].T),
                "mrow": np.ascontiguousarray(m[None, :]),
                "bkrow": np.ascontiguousarray((-B0 * (1.0 - m))[None, :]),
                "abrow": np.ascontiguousarray((B0 * (1.0 - m))[None, :]),
                "bq": (b_proj.astype(np.float32) / 4.0),
            }
        )
    return in_maps


def reduce_outputs(results):
    out = np.zeros((B, N, C), dtype=np.float32)
    for core in range(8):
        out[core // 4] += results[core]["outP"].T
    return out


def kernel(x, attn_mask, w_qkv, w_proj, b_proj, **run_kwargs):
    nc = _get_nc()
    in_maps = make_in_maps(
        np.asarray(x, dtype=np.float32),
        np.asarray(attn_mask),
        np.asarray(w_qkv, dtype=np.float32),
        np.asarray(w_proj, dtype=np.float32),
        np.asarray(b_proj, dtype=np.float32),
    )
    res = run_bass_kernel_spmd(nc, in_maps, core_ids=list(range(8)), **run_kwargs)
    out = reduce_outputs(res.results)
    kernel.last_result = res
    return out


# revision 26
# speedup vs baseline: 1.0136x; 1.0136x over previous
"""Masked multi-head attention (B=2, N=2048, C=1152, H=16, HD=72) on 8 trn2 cores.

Sharding: core c handles batch b = c//4 and heads 4*(c%4) .. 4*(c%4)+4
(data parallel on B, tensor parallel on heads). Each core computes a partial
output projection over its 288 head-channels; the host sums the 4 partials
per batch (TP unshard). b_proj/4 is added on each core so the sum carries the
full bias.

Per-core kernel layout (all fp32, matmuls run as float32r):
  xT   [1152, 2048]  x[b] transposed (contraction dim on partitions)
  qk proj -> QM/KB [73, 4, 2048]: rows 0..71 = per-head qT (mask*scale folded
             in) / kT; row 72 = mask row (q side) / -1e8*(1-mask) (key side).
             Scores matmul then contracts K=73 so the additive mask bias
             bk[k]*m[q] comes out of the PE for free, and masked q columns
             get s=0 -> exp(0)=1 -> uniform attention = reference semantics
             of softmax over an all(-1e8) row.
  v proj  -> V [128, 16, 292] (keys on partitions, 4*(72+1) cols: col 72 of
             each head is 1.0 so the AV matmul also produces the softmax
             denominator).
  scores/exp/AV per (head, 1024-wide q chunk), normalize via DVE reciprocal
             + K=1 PE broadcast matmul, proj accumulates 4 head chunks.
"""

import numpy as np

import concourse.bass as bass
import concourse.mybir as mybir
import concourse.tile as tile
from concourse import bacc
from concourse.bass_utils import run_bass_kernel_spmd

B, N, C = 2, 2048, 1152
H = 16
HD = C // H  # 72
HL = 4  # heads per core
SCALE = HD ** -0.5
B0 = 100007936.0  # fp32r-exact rounding of 1e8, multiple of 8
KD = HD + 1  # 73: contraction + key-bias row
VW = 97  # V slot width per head: v(72) + pad + ones col at 96 (aligned D row)
NT = N // 128  # 16 key chunks
CC = C // 128  # 9 contraction chunks
QW = 1024  # q-chunk width for the exp pass
NQ = N // QW  # 2

F32 = mybir.dt.float32
F32R = mybir.dt.float32r


def build_nc(debug=False):
    nc = bacc.Bacc(None, target_bir_lowering=False)

    xT = nc.dram_tensor("xT", [C, N], F32R, kind="ExternalInput")
    wqkT = nc.dram_tensor("wqkT", [C, 2 * HL * HD], F32R, kind="ExternalInput")
    wvT = nc.dram_tensor("wvT", [C, HL * HD], F32R, kind="ExternalInput")
    wpT = nc.dram_tensor("wpT", [HL * HD, C], F32R, kind="ExternalInput")
    mrow = nc.dram_tensor("mrow", [1, N], F32R, kind="ExternalInput")
    bkrow = nc.dram_tensor("bkrow", [1, N], F32R, kind="ExternalInput")
    abrow = nc.dram_tensor("abrow", [1, N], F32R, kind="ExternalInput")
    bq = nc.dram_tensor("bq", [C], F32, kind="ExternalInput")
    outP = nc.dram_tensor("outP", [C, N], F32, kind="ExternalOutput")
    if debug:
        qm_out = nc.dram_tensor("qm_out", [KD, HL, N], F32, kind="ExternalOutput")
        kb_out = nc.dram_tensor("kb_out", [KD, HL, N], F32, kind="ExternalOutput")
        v_out = nc.dram_tensor("v_out", [128, NT, HL * VW], F32, kind="ExternalOutput")
        ao_out = nc.dram_tensor("ao_out", [HD, HL, N], F32, kind="ExternalOutput")
        a_out = nc.dram_tensor("a_out", [128, QW], F32, kind="ExternalOutput")

    with tile.TileContext(nc) as tc:
        with (
            tc.tile_pool(name="persist", bufs=1) as persist,
        ):
            QM = persist.tile([KD, HL, N], F32R)   # q-side, mask*scale folded
            KB = persist.tile([KD, HL, N], F32R)   # k-side + bias row
            V = persist.tile([128, NT, HL * VW], F32R)
            AO = persist.tile([HD, HL, N], F32R)   # normalized attention out
            ones1 = persist.tile([1, 128], F32R)
            ones1f = persist.tile([1, 128], F32)
            ones128 = persist.tile([128, 1], F32)
            bq_sb = persist.tile([128, CC], F32)

            nc.vector.memset(ones1f, 1.0)
            nc.vector.memset(ones128, 1.0)
            nc.vector.tensor_copy(ones1, ones1f)
            nc.sync.dma_start(bq_sb, bq.rearrange("(o p) -> p o", p=128))
            # bias row 72: QM=m_q, KB=-B0(1-m_k): key-mask bias for unmasked q.
            # Masked-q columns are handled by mm2 (K=1 psum-accumulate of
            # -B0(1-m_q), which quantizes the psum-resident score exactly like
            # the reference's fp32 add of -1e8) + a DVE add-back of +B0(1-m_q)
            # before the exp.
            for h in range(HL):
                nc.sync.dma_start(QM[HD : HD + 1, h, :], mrow[:])
                nc.sync.dma_start(KB[HD : HD + 1, h, :], bkrow[:])
            # V pad + ones columns (cols 72..96 of each head slot) = 1.0
            vtail = V.rearrange("p t (h w) -> p t h w", h=HL)[:, :, :, HD:]
            nc.vector.tensor_copy(
                vtail, ones128.to_broadcast(vtail.shape)
            )

            # ---------- phase 0+1: replicated mask + projections ----------
            with (
                tc.tile_pool(name="ph1", bufs=12) as ph1,
                tc.tile_pool(name="wpool", bufs=1) as wpool,
                tc.tile_pool(name="ps_v", bufs=2, space="PSUM") as ps_v,
                tc.tile_pool(name="ps_qk", bufs=3, space="PSUM") as ps_qk,
            ):
                wqk_sb = wpool.tile([128, CC, 2 * HL * HD], F32R)
                wv_sb = wpool.tile([128, CC, HL * HD], F32R)
                nc.sync.dma_start(
                    wqk_sb, wqkT.rearrange("(o p) m -> p o m", p=128)
                )
                nc.sync.dma_start(wv_sb, wvT.rearrange("(o p) m -> p o m", p=128))

                for nq in range(4):  # 512-token chunks
                    q0 = nq * 512
                    xts = []
                    for cc in range(CC):
                        xt = ph1.tile(
                            [128, 512], F32R, tag="xts", name=f"x_{nq}_{cc}"
                        )
                        nc.sync.dma_start(
                            xt, xT[cc * 128 : (cc + 1) * 128, q0 : q0 + 512]
                        )
                        xts.append(xt)
                    # v projection: tokens on partitions
                    for t in range(4):
                        pv = ps_v.tile([128, HL * HD], F32, tag="pv")
                        for cc in range(CC):
                            nc.tensor.matmul(
                                pv,
                                (xts[cc][:, t * 128 : (t + 1) * 128]),
                                (wv_sb[:, cc, :]),
                                start=(cc == 0),
                                stop=(cc == CC - 1),
                            )
                        kc = nq * 4 + t
                        nc.vector.tensor_copy(
                            V[:, kc, :].rearrange("p (h w) -> p h w", h=HL)[
                                :, :, :HD
                            ],
                            pv.rearrange("p (h w) -> p h w", h=HL),
                        )
                    # q/k projection: channels on partitions, one head per tile
                    for is_q in (True, False):
                        for h in range(HL):
                            r0 = (0 if is_q else HL * HD) + h * HD
                            pqk = ps_qk.tile([HD, 512], F32, tag="pqk")
                            for cc in range(CC):
                                nc.tensor.matmul(
                                    pqk,
                                    (wqk_sb[:, cc, r0 : r0 + HD]),
                                    (xts[cc]),
                                    start=(cc == 0),
                                    stop=(cc == CC - 1),
                                )
                            if is_q:
                                nc.vector.tensor_scalar_mul(
                                    QM[:HD, h, q0 : q0 + 512], pqk, SCALE
                                )
                            else:
                                nc.vector.tensor_copy(
                                    KB[:HD, h, q0 : q0 + 512], pqk
                                )

            # ---------- phase 2: attention ----------
            with (
                tc.tile_pool(name="apool", bufs=4) as apool,
                tc.tile_pool(name="spool", bufs=4) as spool,
                tc.tile_pool(name="small", bufs=2) as small,
                tc.tile_pool(name="ph2row", bufs=1) as ph2row,
                tc.tile_pool(name="ps_s", bufs=2, space="PSUM") as ps_s,
                tc.tile_pool(name="ps_av", bufs=3, space="PSUM") as ps_av,
                tc.tile_pool(name="ps_rep", bufs=1, space="PSUM") as ps_rep,
            ):
                bkrow_sb = ph2row.tile([1, N], F32R)
                abrow_sb = ph2row.tile([1, N], F32R)
                Rqpos = ph2row.tile([128, N], F32)
                nc.sync.dma_start(bkrow_sb, bkrow[:])
                nc.sync.dma_start(abrow_sb, abrow[:])
                for i in range(4):
                    prq = ps_rep.tile([128, 512], F32, tag="prep", name=f"rq{i}", bufs=1)
                    nc.tensor.matmul(
                        prq,
                        ones1[:1, :],
                        abrow_sb[:, i * 512 : (i + 1) * 512],
                        start=True,
                        stop=True,
                    )
                    nc.scalar.copy(Rqpos[:, i * 512 : (i + 1) * 512], prq)
                for h in range(HL):
                    for qc in range(NQ):
                        q0 = qc * QW
                        pavs = [
                            ps_av.tile(
                                [VW, 512], F32, tag="pav", name=f"pav_{h}_{qc}_{i}"
                            )
                            for i in range(2)
                        ]
                        for kc in range(NT):
                            ps = ps_s.tile([128, QW], F32, tag="ps")
                            for qh in range(2):
                                qs = q0 + qh * 512
                                nc.tensor.matmul(
                                    ps[:, qh * 512 : (qh + 1) * 512],
                                    (KB[:, h, kc * 128 : (kc + 1) * 128]),
                                    (QM[:, h, qs : qs + 512]),
                                    start=True,
                                    stop=False,
                                )
                                # quantizer: psum += -B0*(1-m_q) (rounds like ref)
                                nc.tensor.matmul(
                                    ps[:, qh * 512 : (qh + 1) * 512],
                                    ones1[:1, :],
                                    bkrow_sb[:, qs : qs + 512],
                                    start=False,
                                    stop=True,
                                )
                            st = spool.tile([128, QW], F32, tag="st")
                            nc.vector.tensor_tensor(
                                st, ps, Rqpos[:, q0 : q0 + QW],
                                mybir.AluOpType.add,
                            )
                            a = apool.tile([128, QW], F32R, tag="A")
                            nc.scalar.activation(
                                a, st, mybir.ActivationFunctionType.Exp
                            )
                            if debug and h == 0 and qc == 0 and kc == 0:
                                nc.sync.dma_start(a_out[:], a.bitcast(F32))
                            for qh in range(2):
                                nc.tensor.matmul(
                                    pavs[qh],
                                    (V[:, kc, h * VW : (h + 1) * VW]),
                                    (a[:, qh * 512 : (qh + 1) * 512]),
                                    start=(kc == 0),
                                    stop=(kc == NT - 1),
                                )
                        for qh in range(2):
                            qs = q0 + qh * 512
                            rec = small.tile([1, 512], F32R, tag="rec")
                            with nc.allow_low_precision(
                                reason="fp32r matmul input format"
                            ):
                                nc.vector.reciprocal(rec, pavs[qh][96:97, :])
                            prep = ps_rep.tile([HD, 512], F32, tag="prep")
                            nc.tensor.matmul(
                                prep,
                                (ones1[:1, :HD]),
                                (rec),
                                start=True,
                                stop=True,
                            )
                            rep_sb = small.tile([HD, 512], F32, tag="repsb")
                            nc.vector.tensor_copy(rep_sb, prep)
                            nc.vector.tensor_tensor(
                                AO[:, h, qs : qs + 512],
                                pavs[qh][:HD, :],
                                rep_sb,
                                mybir.AluOpType.mult,
                            )

            if debug:
                nc.sync.dma_start(qm_out[:], QM.bitcast(F32))
                nc.sync.dma_start(kb_out[:], KB.bitcast(F32))
                nc.sync.dma_start(v_out[:], V.bitcast(F32))
                nc.sync.dma_start(ao_out[:], AO.bitcast(F32))

            # ---------- phase 3: output projection ----------
            with (
                tc.tile_pool(name="ph3", bufs=1) as ph3,
                tc.tile_pool(name="opool", bufs=3) as opool,
                tc.tile_pool(name="ps_o", bufs=2, space="PSUM") as ps_o,
            ):
                wp_sb = ph3.tile([HD, HL, C], F32R)
                nc.sync.dma_start(
                    wp_sb, wpT.rearrange("(h p) m -> p h m", p=HD)
                )
                for mo in range(CC):
                    for qc in range(4):
                        q0 = qc * 512
                        po = ps_o.tile([128, 512], F32, tag="po")
                        for h in range(HL):
                            nc.tensor.matmul(
                                po,
                                (wp_sb[:, h, mo * 128 : (mo + 1) * 128]),
                                (AO[:, h, q0 : q0 + 512]),
                                start=(h == 0),
                                stop=(h == HL - 1),
                            )
                        ot = opool.tile([128, 512], F32, tag="ot")
                        nc.scalar.add(ot, po, bq_sb[:, mo : mo + 1])
                        nc.sync.dma_start(
                            outP[mo * 128 : (mo + 1) * 128, q0 : q0 + 512], ot
                        )

    nc.compile()
    return nc


_NC = None


def _get_nc():
    global _NC
    if _NC is None:
        _NC = build_nc()
    return _NC


def round_fp32r(a):
    """Round fp32 to the fp32r memory format (mantissa truncated to 11 bits,
    round-to-nearest) — matches walrus's fp32_to_fp32r."""
    b = np.ascontiguousarray(a, dtype=np.float32).view(np.uint32)
    b = ((b.astype(np.uint64) + 0x800) & 0xFFFFF000).astype(np.uint32)
    return b.view(np.float32)


def make_in_maps(x, attn_mask, w_qkv, w_proj, b_proj):
    in_maps = []
    for core in range(8):
        b = core // 4
        hs = (core % 4) * HL  # first head
        qrows = w_qkv[hs * HD : (hs + HL) * HD, :]
        krows = w_qkv[C + hs * HD : C + (hs + HL) * HD, :]
        vrows = w_qkv[2 * C + hs * HD : 2 * C + (hs + HL) * HD, :]
        m = attn_mask[b].astype(np.float32)
        in_maps.append(
            {
                "xT": round_fp32r(x[b].T),
                "wqkT": round_fp32r(np.concatenate([qrows, krows], axis=0).T),
                "wvT": round_fp32r(vrows.T),
                "wpT": round_fp32r(w_proj[:, hs * HD : (hs + HL) * HD].T),
                "mrow": np.ascontiguousarray(m[None, :]),
                "bkrow": np.ascontiguousarray((-B0 * (1.0 - m))[None, :]),
                "abrow": np.ascontiguousarray((B0 * (1.0 - m))[None, :]),
                "bq": (b_proj.astype(np.float32) / 4.0),
            }
        )
    return in_maps


def reduce_outputs(results):
    out = np.zeros((B, N, C), dtype=np.float32)
    for core in range(8):
        out[core // 4] += results[core]["outP"].T
    return out


def kernel(x, attn_mask, w_qkv, w_proj, b_proj, **run_kwargs):
    nc = _get_nc()
    in_maps = make_in_maps(
        np.asarray(x, dtype=np.float32),
        np.asarray(attn_mask),
        np.asarray(w_qkv, dtype=np.float32),
        np.asarray(w_proj, dtype=np.float32),
        np.asarray(b_proj, dtype=np.float32),
    )
    res = run_bass_kernel_spmd(nc, in_maps, core_ids=list(range(8)), **run_kwargs)
    out = reduce_outputs(res.results)
    kernel.last_result = res
    return out
